# revision 90
# baseline (speedup 1.0000x reference)
"""Trainium2 Bass kernel for nn_DecoderAttention (dual-key tree decoder attention).

Sharding: data-parallel over batch B=8, one batch element per NeuronCore.

Per-core computation (B-slice):
  q = target @ Wq + bq                     [T,F]   (qT duplicated on 128 partitions)
  k/v (node, leaf) = x @ {Wk,Wv} + b       (kept transposed [F, *])
  logits = leaf @ Wagg + bagg              [L,1]   (free-1 PE matmuls off leafT)
  Aqn/Aql softmaxes are computed unnormalized (exp, no max-subtraction: |scores/8| <~ 1.2)
  out_pre = (En^T @ [nh|1])/Z1 + (El^T @ [v|1])/Z2 + root/3
  out = softmax_F(out_pre)                 [T,F]
The tree interpolation's root term commutes through the suffix-mean and the
attention average (softmax weights sum to 1), so root/3 is added once at the end.
Suffix cumsum over L: per-128-chunk triangular matmuls (batched 4 chunks / matmul);
cross-chunk carries are added to comb's partition-127 row (the last leaf of each
chunk participates in every in-chunk suffix sum) via a small flatten-DMA.

v2: bf16 datapath (inputs cast in-flight by SWDGE; PE transposes / matmuls at
1 cyc/row; DVE 2x mode on 16-bit copies); the leaf attention (scores -> exp ->
V-accumulate) is fused into the leaf streaming loop so the Activation engine's
exp work overlaps PE transpose/projection work; logits come from free-1 PE
matmuls instead of a Pool multiply + DVE reduce.
"""

import os
import sys

import numpy as np

for _p in ("/opt/trn_rl_repo", "/root/.axon_site/_ro/trn_rl_repo"):
    if os.path.isdir(_p) and _p not in sys.path:
        sys.path.insert(0, _p)

import concourse.bass as bass
import concourse.tile as tile
from concourse import bacc
from concourse import mybir
from concourse.bass_utils import run_bass_kernel_spmd
from concourse.masks import make_identity, make_lower_triangular

FP = mybir.dt.float32
BF = mybir.dt.bfloat16
AF = mybir.ActivationFunctionType
OP = mybir.AluOpType
AX = mybir.AxisListType

B, T, N, L, D, F = 8, 1024, 512, 4096, 512, 64
BR = L // N          # 8 leaves per node
NC = L // 128        # 32 leaf chunks of 128
ND = D // 128        # 4 contraction chunks
SCALE = 1.0 / float(np.sqrt(F))


def _o2_flush(nc, o2_ps, comb, pend, counts):
    """Emit the oldest pending V-accumulation matmul (per-h group of NC)."""
    bchunk, h, el = pend.pop(0)
    NCH = 32  # o2 matmuls per h over the whole stream
    nc.tensor.matmul(o2_ps[h][:], comb[:, bchunk, 0:65], el[:],
                     start=(counts[h] == 0), stop=(counts[h] == NCH - 1),
                     skip_group_check=True)
    counts[h] += 1


def _bcast_ap(ap, parts=128):
    """Partition-broadcast read AP (DRAM sources only)."""
    dims = list(ap.ap)
    if dims and dims[0][1] == 1:
        dims = dims[1:]
    return bass.AP(tensor=ap.tensor, offset=ap.offset, ap=[[0, parts]] + dims)


def _rep_ap(ap, rep):
    """Append a step-0 innermost free dim (read each element `rep` times)."""
    return bass.AP(tensor=ap.tensor, offset=ap.offset, ap=list(ap.ap) + [[0, rep]])


def _flat_ap(ap, n):
    """View an SBUF [p, f] region as a single flat free run (DMA use only)."""
    return bass.AP(tensor=ap.tensor, offset=ap.offset, ap=[[1, n]])


def build_nc():
    nc = bacc.Bacc("TRN2", target_bir_lowering=False, debug=False)

    d_root = nc.dram_tensor("root", [1, F], FP, kind="ExternalInput")
    d_node = nc.dram_tensor("node", [N, D], FP, kind="ExternalInput")
    d_leaf = nc.dram_tensor("leaf", [L, D], FP, kind="ExternalInput")
    d_target = nc.dram_tensor("target", [T, D], FP, kind="ExternalInput")
    d_wq = nc.dram_tensor("Wq", [D, F], FP, kind="ExternalInput")
    d_bq = nc.dram_tensor("bq", [F], FP, kind="ExternalInput")
    d_wk = nc.dram_tensor("Wk", [D, F], FP, kind="ExternalInput")
    d_bk = nc.dram_tensor("bk", [F], FP, kind="ExternalInput")
    d_wv = nc.dram_tensor("Wv", [D, F], FP, kind="ExternalInput")
    d_bv = nc.dram_tensor("bv", [F], FP, kind="ExternalInput")
    d_wagg = nc.dram_tensor("Wagg", [D, 1], FP, kind="ExternalInput")
    d_bagg = nc.dram_tensor("bagg", [1], FP, kind="ExternalInput")
    d_out = nc.dram_tensor("out", [T, F], FP, kind="ExternalOutput")

    with tile.TileContext(nc) as tc:
        _emit(nc, tc, d_root, d_node, d_leaf, d_target, d_wq, d_bq, d_wk, d_bk,
              d_wv, d_bv, d_wagg, d_bagg, d_out)
    nc.compile()
    return nc


def _emit(nc, tc, d_root, d_node, d_leaf, d_target, d_wq, d_bq, d_wk, d_bk,
          d_wv, d_bv, d_wagg, d_bagg, d_out):
    from contextlib import ExitStack

    with ExitStack() as ctx:
        ctx.enter_context(nc.allow_low_precision(
            reason="bf16 datapath; output tolerance is 2e-2 relative"))
        consts = ctx.enter_context(tc.tile_pool(name="consts", bufs=1))
        big = ctx.enter_context(tc.tile_pool(name="big", bufs=1))
        lnat = ctx.enter_context(tc.tile_pool(name="lnat", bufs=4))
        ltp = ctx.enter_context(tc.tile_pool(name="ltp", bufs=2))
        work = ctx.enter_context(tc.tile_pool(name="work", bufs=2))
        epool = ctx.enter_context(tc.tile_pool(name="epool", bufs=6))
        ptr = ctx.enter_context(tc.tile_pool(name="ptr", bufs=2, space="PSUM"))
        pmm = ctx.enter_context(tc.tile_pool(name="pmm", bufs=2, space="PSUM"))
        pst = ctx.enter_context(tc.tile_pool(name="pst", bufs=2, space="PSUM"))
        pacc = ctx.enter_context(tc.tile_pool(name="pacc", bufs=2, space="PSUM"))

        # ---------------- constants ----------------
        ident = consts.tile([128, 128], FP)
        make_identity(nc, ident[:])
        identB = consts.tile([128, 128], BF)
        nc.vector.tensor_copy(identB[:], ident[:])

        tri128 = consts.tile([128, 128], BF)      # [m,l]=1 iff l<=m  (suffix lhsT)
        tri_scr = consts.tile([128, 128], FP)
        make_lower_triangular(nc, tri_scr[:], val=1.0, diag=True)
        nc.vector.tensor_copy(tri128[:], tri_scr[:])
        tri32s = consts.tile([32, 32], BF)        # [k,c]=1 iff k>c   (carry lhsT)
        tri32_scr = consts.tile([32, 32], FP)
        make_lower_triangular(nc, tri32_scr[:], val=1.0, diag=False)
        nc.vector.tensor_copy(tri32s[:], tri32_scr[:])

        # G[m,j] = 1 iff m//8 == j  (leaf->node group indicator), GT transposed
        Gf = consts.tile([128, 16], FP)
        nc.gpsimd.memset(Gf[:], 1.0)
        nc.gpsimd.affine_select(out=Gf[:], in_=Gf[:], compare_op=OP.is_ge, fill=0.0,
                                base=0, pattern=[[-BR, 16]], channel_multiplier=1)
        nc.gpsimd.affine_select(out=Gf[:], in_=Gf[:], compare_op=OP.is_ge, fill=0.0,
                                base=BR - 1, pattern=[[BR, 16]], channel_multiplier=-1)
        G = consts.tile([128, 16], BF)
        nc.vector.tensor_copy(G[:], Gf[:])
        GTf = consts.tile([16, 128], FP)
        nc.gpsimd.memset(GTf[:], 1.0)
        nc.gpsimd.affine_select(out=GTf[:], in_=GTf[:], compare_op=OP.is_ge, fill=0.0,
                                base=0, pattern=[[1, 128]], channel_multiplier=-BR)
        nc.gpsimd.affine_select(out=GTf[:], in_=GTf[:], compare_op=OP.is_ge, fill=0.0,
                                base=BR - 1, pattern=[[-1, 128]], channel_multiplier=BR)
        GT = consts.tile([16, 128], BF)
        nc.vector.tensor_copy(GT[:], GTf[:])

        ones_scr = consts.tile([128, 64], FP)
        nc.gpsimd.memset(ones_scr[:], 1.0)
        onesB = consts.tile([128, 64], BF)
        nc.vector.tensor_copy(onesB[:], ones_scr[:])

        # 1 / (3 * (L - l)) with l = 128*c + p   -> [128, 32]
        cnt3 = consts.tile([128, NC], FP)
        nc.gpsimd.iota(cnt3[:], pattern=[[-3 * 128, NC]], base=3 * L,
                       channel_multiplier=-3, allow_small_or_imprecise_dtypes=True)
        inv3 = consts.tile([128, NC], FP)
        nc.vector.reciprocal(inv3[:], cnt3[:])

        # ---------------- weights / biases (bf16 working copies, cast on ACT) --
        w_kv = consts.tile([128, ND, 128], BF)     # cols 0:64 Wk, 64:128 Wv
        w_qq = consts.tile([128, ND, 128], BF)     # Wq duplicated
        wk_raw = consts.tile([128, ND, F], FP)
        wv_raw = consts.tile([128, ND, F], FP)
        wq_raw = consts.tile([128, ND, F], FP)
        nc.sync.dma_start(wk_raw[:], d_wk[:].rearrange("(j p) f -> p j f", p=128))
        nc.sync.dma_start(wv_raw[:], d_wv[:].rearrange("(j p) f -> p j f", p=128))
        nc.sync.dma_start(wq_raw[:], d_wq[:].rearrange("(j p) f -> p j f", p=128))
        for dc in range(ND):
            nc.scalar.activation(out=w_kv[:, dc, 0:F], in_=wk_raw[:, dc, :],
                                 func=AF.Copy)
            nc.scalar.activation(out=w_kv[:, dc, F:128], in_=wv_raw[:, dc, :],
                                 func=AF.Copy)
            nc.scalar.activation(out=w_qq[:, dc, 0:F], in_=wq_raw[:, dc, :],
                                 func=AF.Copy)
            nc.scalar.activation(out=w_qq[:, dc, F:128], in_=wq_raw[:, dc, :],
                                 func=AF.Copy)

        # Wagg as matmul rhs: [p, j] = Wagg[j*128+p]
        wagg_rhs = consts.tile([128, ND], BF)
        wagg_raw = consts.tile([128, ND], FP)
        nc.sync.dma_start(wagg_raw[:],
                          d_wagg[:].rearrange("(j p) o -> p (j o)", p=128))
        nc.scalar.activation(out=wagg_rhs[:], in_=wagg_raw[:], func=AF.Copy)

        # bias loads on HWDGE (sync) — SWDGE's 1us fixed overhead per DMA would
        # stall the Pool queue ahead of the big cast-loads
        bias_q = consts.tile([128, 1], FP)
        bias_k = consts.tile([128, 1], FP)
        bias_v = consts.tile([128, 1], FP)
        bq2 = d_bq[:].rearrange("(f o) -> f o", o=1)
        bk2 = d_bk[:].rearrange("(f o) -> f o", o=1)
        bv2 = d_bv[:].rearrange("(f o) -> f o", o=1)
        nc.sync.dma_start(bias_q[0:F, :], bq2)
        nc.sync.dma_start(bias_q[F:128, :], bq2)
        nc.sync.dma_start(bias_k[0:F, :], bk2)
        nc.sync.dma_start(bias_k[F:128, :], bk2)
        nc.sync.dma_start(bias_v[0:F, :], bv2)
        bagg_b = consts.tile([128, 1], FP)
        nc.sync.dma_start(bagg_b[:], _bcast_ap(d_bagg[:]))

        # rootT3 = root^T / 3   [64, 1]
        root_row = consts.tile([1, F], FP)
        nc.sync.dma_start(root_row[:], d_root[:])
        rt_ps = pmm.tile([F, 1], FP, tag="mm")
        nc.tensor.transpose(rt_ps[:], root_row[:], ident[0:1, 0:1])
        rootT3 = consts.tile([F, 1], FP)
        nc.scalar.activation(out=rootT3[:], in_=rt_ps[:], func=AF.Copy, scale=1.0 / 3.0)

        # leaf loads for blocks 0/1 are interleaved between the target loads so
        # the leaf stream can start while the q/node prologue still runs
        lns = {}

        def _load_leaf(i):
            lns[i] = lnat.tile([128, 4, D], BF, tag="xnat", name=f"ln{i}")
            nc.gpsimd.dma_start(lns[i][:], d_leaf[i * 512:(i + 1) * 512, :]
                                .rearrange("(j p) d -> p j d", p=128))

        # ---------------- target -> qdual [128, 1024] (bf16) ----------------
        targT = big.tile([128, ND, T], BF)
        for ib in range(T // 512):
            tn = lnat.tile([128, 4, D], BF, tag="xnat")
            nc.gpsimd.dma_start(tn[:], d_target[ib * 512:(ib + 1) * 512, :]
                                .rearrange("(j p) d -> p j d", p=128))
            for j in range(4):
                i = 4 * ib + j
                tp = ptr.tile([128, 512], BF, tag="tp")
                for dc in range(ND):
                    nc.tensor.transpose(tp[:, dc * 128:(dc + 1) * 128],
                                        tn[:, j, dc * 128:(dc + 1) * 128], identB[:])
                nc.vector.tensor_copy(
                    targT[:, 0:ND, i * 128:(i + 1) * 128],
                    tp[:].rearrange("p (dc b) -> p dc b", b=128))
        qdual = big.tile([128, T], BF)
        for h in range(2):
            q_ps = pmm.tile([128, 512], FP, tag="mm")
            for dc in range(ND):
                nc.tensor.matmul(q_ps[:], w_qq[:, dc, :],
                                 targT[:, dc, h * 512:(h + 1) * 512],
                                 start=(dc == 0), stop=(dc == ND - 1))
            nc.scalar.activation(out=qdual[:, h * 512:(h + 1) * 512], in_=q_ps[:],
                                 func=AF.Identity, bias=bias_q[:])

        # ---------------- node -> kTn_dual [128, 256], node_vT [64, 512] -------
        nodeT = big.tile([128, ND, N], BF)
        nn = lnat.tile([128, 4, D], BF, tag="xnat")
        nc.gpsimd.dma_start(nn[:], d_node[:].rearrange("(j p) d -> p j d", p=128))
        for i in range(N // 128):
            tp = ptr.tile([128, 512], BF, tag="tp")
            for dc in range(ND):
                nc.tensor.transpose(tp[:, dc * 128:(dc + 1) * 128],
                                    nn[:, i, dc * 128:(dc + 1) * 128], identB[:])
            nc.vector.tensor_copy(nodeT[:, 0:ND, i * 128:(i + 1) * 128],
                                  tp[:].rearrange("p (dc b) -> p dc b", b=128))
        kTn_dual = big.tile([128, 256], BF)
        node_vT = big.tile([64, N], BF)
        kvn_ps = pmm.tile([128, 512], FP, tag="mm")
        for dc in range(ND):
            nc.tensor.matmul(kvn_ps[:], w_kv[:, dc, :], nodeT[:, dc, :],
                             start=(dc == 0), stop=(dc == ND - 1))
        for b in range(4):
            ro, co = (b % 2) * 64, (b // 2) * 128
            nc.scalar.activation(out=kTn_dual[ro:ro + 64, co:co + 128],
                                 in_=kvn_ps[0:64, b * 128:(b + 1) * 128],
                                 func=AF.Identity, bias=bias_k[ro:ro + 64, :])
        nc.scalar.activation(out=node_vT[:], in_=kvn_ps[64:128, :],
                             func=AF.Identity, bias=bias_v[0:64, :])

        # node scores + exp early: fills ACT while PE transposes leaf blocks;
        # o1 accumulation happens after node_hat is ready.
        en_sb = big.tile([128, 4, T], BF)
        for b in range(4):
            ro, co = (b % 2) * 64, (b // 2) * 128
            for h in range(2):
                st = pst.tile([128, 512], FP, tag="st")
                nc.tensor.matmul(st[:], kTn_dual[ro:ro + 64, co:co + 128],
                                 qdual[ro:ro + 64, h * 512:(h + 1) * 512],
                                 start=True, stop=True)
                nc.scalar.activation(out=en_sb[:, b, h * 512:(h + 1) * 512],
                                     in_=st[:], func=AF.Exp, scale=SCALE)

        # ---------------- leaf stream: transposes, kv, logits, attention -------
        kTdual = big.tile([128, L // 2], BF)   # block i -> rows (i%2)*64, cols (i//2)*512
        tile12 = big.tile([128, L], BF)        # rows 0:64 leaf_vT, rows 64:128 interp'T
        comb = big.tile([128, NC, 129], BF)    # [v(64) | ones | interp(64)]
        nc.vector.tensor_copy(comb[:, :, 64:65],
                              ones_scr[:, 0:NC].rearrange("p (c o) -> p c o", o=1))
        e_all = big.tile([128, NC], BF)
        o2_ps = [pacc.tile([65, 512], FP, tag="oacc", name=f"o2_ps{h}")
                 for h in range(2)]
        o2_pend = []
        o2_first = [0, 0]   # per-h count of emitted o2 accumulation matmuls
        # node-hat weight slabs: chunk c writes G*w_all[:,c] at columns
        # 16*(c%8) of slab c; everything else stays zero (no restores needed)
        wbig = big.tile([128, NC, 128], BF)
        nc.vector.memset(wbig[:], 0.0)
        w_all = big.tile([128, NC], FP)
        totT = big.tile([64, NC], BF)

        att_q = []    # deferred attention emitters, drained between chunk work

        def _att_drain(k=1):
            for _ in range(k):
                if att_q:
                    att_q.pop(0)()

        def _comb_and_att(ib):
            """comb transposes/copies for block ib (one merged copy per chunk:
            tp cols [0:64 | 64:128] -> comb cols [0:64 | 65:129]), then enqueue
            the block's 8 attention slots (its own kTdual row-half)."""
            for j in range(4):
                c = 4 * ib + j
                tp = ptr.tile([128, 512], BF, tag="tp")
                nc.tensor.transpose(tp[:, 0:128], tile12[:, c * 128:(c + 1) * 128],
                                    identB[:])
                cdst = comb[:, c, 0:64]
                cdst2 = bass.AP(tensor=cdst.tensor, offset=cdst.offset,
                                ap=[list(cdst.ap[0]), [65, 2], [1, 64]])
                nc.vector.tensor_copy(cdst2,
                                      tp[:, 0:128].rearrange("p (g x) -> p g x", g=2))
                _att_drain(1)
            ro2 = (ib % 2) * 64
            for j in range(4):
                cchunk = slice((ib // 2) * 512 + j * 128,
                               (ib // 2) * 512 + (j + 1) * 128)
                bchunk = 4 * ib + j
                for h in range(2):
                    def emit_att(cchunk=cchunk, ro2=ro2, bchunk=bchunk, h=h):
                        st = pst.tile([128, 512], FP, tag="st")
                        nc.tensor.matmul(st[:], kTdual[ro2:ro2 + 64, cchunk],
                                         qdual[ro2:ro2 + 64,
                                               h * 512:(h + 1) * 512],
                                         start=True, stop=True)
                        el = epool.tile([128, 512], BF, tag="el")
                        nc.scalar.activation(out=el[:], in_=st[:],
                                             func=AF.Exp, scale=SCALE)
                        o2_pend.append((bchunk, h, el))
                        if len(o2_pend) > 2:
                            _o2_flush(nc, o2_ps, comb, o2_pend, o2_first)
                    att_q.append(emit_att)

        _load_leaf(0)
        for i in range(L // 512):
            if i + 1 < L // 512:
                _load_leaf(i + 1)
            leafT = ltp.tile([128, ND, 512], BF)
            ln = lns.pop(i)
            for j in range(4):
                tp = ptr.tile([128, 512], BF, tag="tp")
                for dc in range(ND):
                    nc.tensor.transpose(tp[:, dc * 128:(dc + 1) * 128],
                                        ln[:, j, dc * 128:(dc + 1) * 128], identB[:])
                # alternate the PSUM->SBUF copies between DVE and ACT to
                # balance per-block engine load
                if j % 2 == 0:
                    nc.vector.tensor_copy(leafT[:, 0:ND, j * 128:(j + 1) * 128],
                                          tp[:].rearrange("p (dc b) -> p dc b", b=128))
                else:
                    nc.scalar.activation(
                        out=leafT[:, 0:ND, j * 128:(j + 1) * 128],
                        in_=tp[:].rearrange("p (dc b) -> p dc b", b=128),
                        func=AF.Copy)
                _att_drain(1)
            # logits for this block's 4 chunks: free-1 accumulating matmuls
            lg_ps = pmm.tile([128, 4], FP, tag="mm")
            for j in range(4):
                for dc in range(ND):
                    nc.tensor.matmul(lg_ps[:, j:j + 1],
                                     leafT[:, dc, j * 128:(j + 1) * 128],
                                     wagg_rhs[:, dc:dc + 1],
                                     start=(dc == 0), stop=(dc == ND - 1),
                                     skip_group_check=True)
            _att_drain(1)
            nc.scalar.activation(out=e_all[:, 4 * i:4 * i + 4], in_=lg_ps[:],
                                 func=AF.Exp, bias=bagg_b[:])
            # group softmax weights for these 4 chunks (groups are intra-chunk)
            s4 = pmm.tile([16, 4], FP, tag="mm")
            nc.tensor.matmul(s4[:], G[:], e_all[:, 4 * i:4 * i + 4],
                             start=True, stop=True)
            sinv4 = work.tile([16, 4], BF, tag="sinv")
            nc.vector.reciprocal(sinv4[:], s4[:])
            r4 = pmm.tile([128, 4], FP, tag="mm")
            nc.tensor.matmul(r4[:], GT[:], sinv4[:], start=True, stop=True)
            nc.vector.tensor_tensor(out=w_all[:, 4 * i:4 * i + 4],
                                    in0=e_all[:, 4 * i:4 * i + 4], in1=r4[:],
                                    op=OP.mult)
            for j in range(4):
                c = 4 * i + j
                bo = 16 * (c % 8)
                nc.vector.tensor_scalar(out=wbig[:, c, bo:bo + 16],
                                        in0=G[:], scalar1=w_all[:, c:c + 1],
                                        scalar2=None, op0=OP.mult)
            # dual k/v projection
            kv_ps = pmm.tile([128, 512], FP, tag="mm")
            for dc in range(ND):
                nc.tensor.matmul(kv_ps[:], w_kv[:, dc, :], leafT[:, dc, :],
                                 start=(dc == 0), stop=(dc == ND - 1))
            _att_drain(1)
            ro, co = (i % 2) * 64, (i // 2) * 512
            # kT write on DVE (ACT is busy with exps), v write on ACT
            nc.vector.tensor_scalar(out=kTdual[ro:ro + 64, co:co + 512],
                                    in0=kv_ps[0:64, :],
                                    scalar1=bias_k[ro:ro + 64, :], scalar2=None,
                                    op0=OP.add)
            _att_drain(1)
            sl = slice(i * 512, (i + 1) * 512)
            nc.vector.tensor_scalar(out=tile12[0:64, sl], in0=kv_ps[64:128, :],
                                    scalar1=bias_v[0:64, :], scalar2=None,
                                    op0=OP.add)
            _att_drain(1)
            # interp'T = leaf_vT + node_vT replicated 8x along l (no root, no /3)
            base = node_vT[0:64, 64 * i:64 * (i + 1)]
            nc.vector.tensor_tensor(
                out=tile12[64:128, sl].rearrange("f (n c) -> f n c", c=BR),
                in0=tile12[0:64, sl].rearrange("f (n c) -> f n c", c=BR),
                in1=_rep_ap(base, BR), op=OP.add)
            _att_drain(1)
            # per-chunk interp totals (feeds the cross-chunk carries)
            nc.vector.tensor_reduce(
                out=totT[:, 4 * i:4 * i + 4],
                in_=tile12[64:128, sl].rearrange("f (c m) -> f c m", m=128),
                axis=AX.X, op=OP.add)
            _att_drain(1)
            # comb (natural v/interp) for the PREVIOUS block: one block of lag
            # means its transposes never wait on this block's interp chain
            if i > 0:
                _comb_and_att(i - 1)
        _comb_and_att(L // 512 - 1)

        # ---- tail: carries happen between attention drains; the o2-side comb
        # columns are disjoint from the interp columns the carry DMA touches
        tot_ps = ptr.tile([NC, 64], BF, tag="tp")
        nc.tensor.transpose(tot_ps[:], totT[:], identB[0:64, 0:64])
        totals = work.tile([NC, 64], BF, tag="tot")
        nc.vector.tensor_copy(totals[:], tot_ps[:])
        # carrN[c, f] = sum_{k>c} totals[k, f]
        carrN_ps = pmm.tile([NC, 64], FP, tag="mm")
        nc.tensor.matmul(carrN_ps[:], tri32s[:], totals[:], start=True, stop=True)
        carrN = work.tile([NC, 64], BF, tag="carrN")
        nc.vector.tensor_copy(carrN[:], carrN_ps[:])

        while att_q:
            _att_drain(1)
        while o2_pend:
            _o2_flush(nc, o2_ps, comb, o2_pend, o2_first)

        # accumulate into comb[127, c, 65+f] (partition-127 writes need DMA:
        # compute engines can't address a partition range starting at 127)
        nc.gpsimd.dma_start(comb[127:128, :, 65:129], carrN[:], accum_op=OP.add)

        # free o2 accumulator banks before o1 reuses the pacc pool; kick the
        # Z2 reciprocals immediately
        o1_sb = big.tile([65, T], FP)
        o2_sb = big.tile([65, T], FP)
        fs1 = work.tile([65, T], BF, tag="fs")
        fs2 = work.tile([65, T], BF, tag="fs")
        for h in range(2):
            hs = slice(h * 512, (h + 1) * 512)
            nc.scalar.activation(out=o2_sb[:, hs], in_=o2_ps[h][:], func=AF.Copy)
            nc.vector.reciprocal(fs2[64:65, hs], o2_sb[64:65, hs])

        # ---------------- suffix-mean (4 chunks per matmul) + node_hat ---------
        nh_nat = big.tile([128, 4, 65], BF)
        nc.vector.tensor_copy(nh_nat[:, :, 64:65],
                              ones_scr[:, 0:4].rearrange("p (c o) -> p c o", o=1))
        for c4 in range(NC // 4):
            sfx_ps = pst.tile([128, 4, 64], FP, tag="st")
            nc.tensor.matmul(sfx_ps[:], tri128[:], comb[:, 4 * c4:4 * c4 + 4, 65:129],
                             start=True, stop=True)
            upw4 = work.tile([128, 4, 64], BF, tag="upw")
            nc.vector.tensor_tensor(out=upw4[:], in0=sfx_ps[:],
                                    in1=_rep_ap(inv3[:, 4 * c4:4 * c4 + 4], 64),
                                    op=OP.mult)
            for jc in range(4):
                c = 4 * c4 + jc
                if c % 8 == 0:
                    nh_ps = pmm.tile([128, 64], FP, tag="mm", name=f"nh_ps{c // 8}")
                nc.tensor.matmul(nh_ps[:], wbig[:, c, :], upw4[:, jc, :],
                                 start=(c % 8 == 0), stop=(c % 8 == 7),
                                 skip_group_check=True)
                if c % 8 == 7:
                    nc.scalar.activation(out=nh_nat[:, c // 8, 0:64], in_=nh_ps[:],
                                         func=AF.Copy)

        # ---------------- node attention accumulation -> o1 [65, 1024] ---------
        o1_ps = [pacc.tile([65, 512], FP, tag="oacc", name=f"o1_ps{h}")
                 for h in range(2)]
        for b in range(4):
            for h in range(2):
                nc.tensor.matmul(o1_ps[h][:], nh_nat[:, b, :],
                                 en_sb[:, b, h * 512:(h + 1) * 512],
                                 start=(b == 0), stop=(b == 3),
                                 skip_group_check=True)
        for h in range(2):
            hs = slice(h * 512, (h + 1) * 512)
            nc.scalar.activation(out=o1_sb[:, hs], in_=o1_ps[h][:], func=AF.Copy)
            nc.vector.reciprocal(fs1[64:65, hs], o1_sb[64:65, hs])

        # ---------------- combine + final softmax over F (bf16) ----------------
        outT = big.tile([64, T], BF)
        # staggered 2-half pipeline: h=1 runs one stage behind h=0 so PE, DVE
        # and ACT always have the other half's work while a dependency drains
        HS = [slice(0, 512), slice(512, 1024)]
        b1, b2, x1, x2, s12, pre, e3, z3, b3 = ({} for _ in range(9))

        def fb_bcast(h):
            b1[h] = pmm.tile([64, 512], FP, tag="mm", name=f"b1_{h}")
            nc.tensor.matmul(b1[h][:], onesB[64:65, 0:64], fs1[64:65, HS[h]],
                             start=True, stop=True)
            b2[h] = pmm.tile([64, 512], FP, tag="mm", name=f"b2_{h}")
            nc.tensor.matmul(b2[h][:], onesB[64:65, 0:64], fs2[64:65, HS[h]],
                             start=True, stop=True)

        def fb_x(h):
            x1[h] = work.tile([64, 512], BF, tag="x1", name=f"x1_{h}")
            nc.vector.tensor_tensor(out=x1[h][:], in0=o1_sb[0:64, HS[h]],
                                    in1=b1[h][:], op=OP.mult)
            x2[h] = work.tile([64, 512], BF, tag="x2", name=f"x2_{h}")
            nc.vector.tensor_tensor(out=x2[h][:], in0=o2_sb[0:64, HS[h]],
                                    in1=b2[h][:], op=OP.mult)

        def fb_pre(h):
            s12[h] = work.tile([64, 512], BF, tag="s12", name=f"s12_{h}")
            nc.vector.tensor_tensor(out=s12[h][:], in0=x1[h][:], in1=x2[h][:],
                                    op=OP.add)
            pre[h] = work.tile([64, 512], BF, tag="pre", name=f"pre_{h}")
            nc.vector.tensor_scalar(out=pre[h][:], in0=s12[h][:], scalar1=rootT3[:],
                                    scalar2=None, op0=OP.add)

        def fb_exp(h):
            e3[h] = work.tile([64, 512], BF, tag="e3", name=f"e3_{h}")
            nc.scalar.activation(out=e3[h][:], in_=pre[h][:], func=AF.Exp)

        def fb_z(h):
            z3[h] = pmm.tile([1, 512], FP, tag="mm", name=f"z3_{h}")
            nc.tensor.matmul(z3[h][:], onesB[0:64, 0:1], e3[h][:],
                             start=True, stop=True)

        def fb_recip(h):
            nc.vector.reciprocal(fs1[0:1, HS[h]], z3[h][:])

        def fb_b3(h):
            b3[h] = pmm.tile([64, 512], FP, tag="mm", name=f"b3_{h}")
            nc.tensor.matmul(b3[h][:], onesB[0:1, 0:64], fs1[0:1, HS[h]],
                             start=True, stop=True)

        def fb_out(h):
            nc.vector.tensor_tensor(out=outT[:, HS[h]], in0=e3[h][:], in1=b3[h][:],
                                    op=OP.mult)

        stages = [fb_bcast, fb_x, fb_pre, fb_exp, fb_z, fb_recip, fb_b3, fb_out]
        for k in range(len(stages) + 1):
            if k < len(stages):
                stages[k](0)
            if k >= 1:
                stages[k - 1](1)

        onat = big.tile([128, T // 128, F], FP)
        d_out_v = d_out[:].rearrange("(k p) f -> p k f", p=128)
        for k2 in range(T // 256):
            op_ = ptr.tile([128, 512], BF, tag="tp")
            for k in (2 * k2, 2 * k2 + 1):
                nc.tensor.transpose(op_[:, (k % 2) * 64:(k % 2) * 64 + 64],
                                    outT[:, k * 128:(k + 1) * 128],
                                    identB[0:64, 0:64])
            nc.vector.tensor_copy(onat[:, 2 * k2:2 * k2 + 2, :]
                                  .rearrange("p k f -> p (k f)"), op_[:, 0:128])
            # store each 256-target slab as soon as it is ready
            nc.sync.dma_start(d_out_v[:, 2 * k2:2 * k2 + 2, :],
                              onat[:, 2 * k2:2 * k2 + 2, :])


_NC_CACHE = None


def kernel(**inputs):
    global _NC_CACHE
    if _NC_CACHE is None:
        _NC_CACHE = build_nc()
    nc = _NC_CACHE
    shared = {k: np.ascontiguousarray(np.asarray(inputs[k], dtype=np.float32))
              for k in ("Wq", "bq", "Wk", "bk", "Wv", "bv", "Wagg", "bagg")}
    in_maps = []
    for b in range(B):
        m = dict(shared)
        m["root"] = np.ascontiguousarray(np.asarray(inputs["root"][b], dtype=np.float32))
        m["node"] = np.ascontiguousarray(np.asarray(inputs["node"][b], dtype=np.float32))
        m["leaf"] = np.ascontiguousarray(np.asarray(inputs["leaf"][b], dtype=np.float32))
        m["target"] = np.ascontiguousarray(np.asarray(inputs["target"][b], dtype=np.float32))
        in_maps.append(m)
    res = run_bass_kernel_spmd(nc, in_maps, core_ids=list(range(B)))
    return np.stack([r["out"] for r in res.results], axis=0)
